# revision 6
# baseline (speedup 1.0000x reference)
"""Trainium2 Bass kernel for nn_AlignmentMatrix.

Math (per batch b):
    out[b,i,j] = ctx[b]@w1 [i] + asp[b]@w2 [j] + (ctx[b]*w3) @ asp[b].T [i,j]
with ctx [B,L1,H2]=[128,1024,600], asp [B,L2,H2]=[128,128,600],
w_u=[w1;w2;w3] each [600].

Device-side formulation (all FLOPs on device):
    rhs'[d,j]  = w3[d]*asp[b,j,d] + w1[d]        (DVE scale/bias, folds s_ctx;
                                                  batched over all b per chunk)
    s_asp[j]   = sum_d asp[b,j,d]*w2[d]          (PE, lhsT=asp chunk, rhs=w2
                                                  column -> PSUM column [L2,1])
    out_ps[j,i]= sum_d rhs'[d,j]*ctxT[d,i]       (PE, 5 K-chunks of 120)
    out[j,i]   = fp16(out_ps[j,i] + s_asp[j])    (DVE copy w/ per-partition
                                                  scalar add -> free rank-1)

The kernel is HBM-DMA-bound (SDMA read path ~17.5 GB/s/engine,
latency-bound), so BYTES are time:
  * mixed-precision ctx: contraction dims sorted by (w3^2+w1^2) on the
    host (permutation-invariant contraction; PE takes the fp8 moving
    operand against the fp16 stationary rhs'), top 240 dims fp16, rest
    fp8-e4m3.  Measured end-to-end rel err 1.1e-2 vs the 2e-2 gate.
  * reads AND writes spread greedily (byte-balanced) over all three
    dynamic DMA queues (sync/scalar HWDGE + gpsimd SWDGE);
  * ctx streamed in batch PAIRS (8KB/6KB descriptors, half the DMAs).
The pipeline is kept ahead of the DMA ring so engines never starve:
prefetch 3 pairs, 6 PSUM banks for the output, s_asp matmuls hoisted
into per-part bursts, and a dummy-matmul warmup keeps the PE HAM at
K=8/8 (a cold PE slows batch turnover, which throttles the SBUF
buffer recycle that feeds the DMA rings).  fp32 accumulation in PSUM.

Sharding: data-parallel over batch, 16 batches per core across 8 cores.
"""

import numpy as np
import ml_dtypes

import concourse.bass as bass
import concourse.bacc as bacc
import concourse.mybir as mybir
import concourse.tile as tile
from concourse.bass_utils import run_bass_kernel_spmd

N_CORES = 8
B = 128
L1 = 1024  # ctx rows (i)
L2 = 128  # asp rows (j)
H = 600  # contraction dim (d)
BPC = B // N_CORES  # batches per core
KC = 5  # contraction chunks
KP = H // KC  # 120 rows per chunk
KC16 = 2  # leading chunks kept fp16 (largest |w3|,|w1| dims)
KC8 = KC - KC16  # trailing chunks in fp8-e4m3
NI = 512  # moving free-dim per matmul (one PSUM bank of fp32)
NIC = L1 // NI  # i-chunks per batch
OPACK = 2  # batches packed per output DMA
PFP = 3  # ctx prefetch distance in PAIRS (bufs = PFP + 1)
NWARM = 40  # dummy matmuls to bring the PE HAM to K=8/8 before work lands

F32 = mybir.dt.float32
F16 = mybir.dt.float16
F8 = mybir.dt.float8e4
NP_F8 = ml_dtypes.float8_e4m3

CTX16_B = KP * 2 * KC16 * L1 * 2  # bytes of one PAIR of fp16 ctx chunks
CTX8_B = KP * 2 * KC8 * L1 * 1  # bytes of one PAIR of fp8 ctx chunks
ASP_B = KP * KC * L2 * 2  # bytes of one batch of asp
OUT_B = L2 * OPACK * L1 * 2  # bytes of one output pair write

ASP_PARTS = [(0, 2), (2, 6), (6, 10), (10, BPC)]


def build_kernel():
    nc = bacc.Bacc(
        "TRN2", target_bir_lowering=False, debug=False, enable_asserts=False
    )
    ctxT16 = nc.dram_tensor(
        "ctxT16", [BPC // 2, KP, 2, KC16, L1], F16, kind="ExternalInput"
    ).ap()
    ctxT8 = nc.dram_tensor(
        "ctxT8", [BPC // 2, KP, 2, KC8, L1], F8, kind="ExternalInput"
    ).ap()
    aspT = nc.dram_tensor(
        "aspT", [KP, BPC, KC, L2], F16, kind="ExternalInput"
    ).ap()
    wc = nc.dram_tensor("wc", [KP, 2 * KC], F32, kind="ExternalInput").ap()
    w2c = nc.dram_tensor("w2c", [KP, KC], F16, kind="ExternalInput").ap()
    outT = nc.dram_tensor(
        "outT", [BPC // OPACK, L2, OPACK, L1], F16, kind="ExternalOutput"
    ).ap()

    # All three dynamic DMA queues, greedy byte-balanced.
    dmae = [nc.sync, nc.scalar, nc.gpsimd]
    qbytes = [0, 0, 0]

    def q_issue(dst, src, nbytes, force=None):
        i = force if force is not None else min(range(3), key=lambda j: qbytes[j])
        qbytes[i] += nbytes
        dmae[i].dma_start(dst, src)

    MUL = mybir.AluOpType.mult
    ADD = mybir.AluOpType.add

    with tile.TileContext(nc) as tc:
        with (
            tc.tile_pool(name="consts", bufs=1) as consts,
            tc.tile_pool(name="c16_pool", bufs=PFP + 1) as c16_pool,
            tc.tile_pool(name="c8_pool", bufs=PFP + 1) as c8_pool,
            tc.tile_pool(name="asp_pool", bufs=1) as asp_pool,
            tc.tile_pool(name="rhsp_pool", bufs=1) as rhsp_pool,
            tc.tile_pool(name="sasp_pool", bufs=1) as sasp_pool,
            tc.tile_pool(name="out_pool", bufs=3) as out_pool,
            tc.tile_pool(name="ps_out", bufs=6, space="PSUM") as ps_out,
            tc.tile_pool(name="ps_sasp", bufs=2, space="PSUM") as ps_sasp,
        ):
            wc_t = consts.tile([KP, 2 * KC], F32)
            w2c_t = consts.tile([KP, KC], F16)
            asp_t = asp_pool.tile([KP, BPC, KC, L2], F16)
            rhsp_t = rhsp_pool.tile([KP, BPC, KC, L2], F16)
            sasp_t = sasp_pool.tile([L2, BPC], F32)
            dummy = consts.tile([KP, NI], F16)

            ctx_tiles = {}

            def load_pair(p, qa=None, qb=None):
                t16 = c16_pool.tile([KP, 2, KC16, L1], F16, tag="c16")
                t8 = c8_pool.tile([KP, 2, KC8, L1], F8, tag="c8")
                q_issue(t16[:], ctxT16[p], CTX16_B, force=qa)
                q_issue(t8[:], ctxT8[p], CTX8_B, force=qb)
                ctx_tiles[p] = (t16, t8)

            def load_asp(g, force=None):
                lo, hi = ASP_PARTS[g]
                q_issue(
                    asp_t[:, lo:hi], aspT[:, lo:hi], (hi - lo) * ASP_B, force=force
                )

            def rhsp_round(g):
                # rhs'[d,b,k,j] = w3[d,k]*asp[d,b,k,j] + w1[d,k], one DVE op
                # per chunk k batched over the part's batches.
                lo, hi = ASP_PARTS[g]
                for k in range(KC):
                    nc.vector.tensor_scalar(
                        rhsp_t[:, lo:hi, k, :],
                        asp_t[:, lo:hi, k, :],
                        wc_t[:, KC + k : KC + k + 1],
                        wc_t[:, k : k + 1],
                        MUL,
                        ADD,
                    )

            def sasp_burst(g):
                # s_asp columns for the whole part: keeps the in-loop PE
                # work to the main matmuls only.
                lo, hi = ASP_PARTS[g]
                for b in range(lo, hi):
                    sasp_ps = ps_sasp.tile([L2, 1], F32, tag="sasp")
                    for k in range(KC):
                        nc.tensor.matmul(
                            sasp_ps[:],
                            asp_t[:, b, k, :],
                            w2c_t[:, k : k + 1],
                            start=(k == 0),
                            stop=(k == KC - 1),
                        )
                    nc.vector.tensor_copy(sasp_t[:, b : b + 1], sasp_ps[:])

            # Startup: the three big first reads hit three distinct queues
            # immediately; the tiny const loads ride behind them on sync.
            load_pair(0, qa=0, qb=1)
            load_asp(0, force=2)
            nc.sync.dma_start(w2c_t[:], w2c[:])
            nc.sync.dma_start(wc_t[:], wc[:])

            # PE warmup on dummy data: HAM un-throttles after ~3.4us of
            # activity, so the first real batches run at 2.4 GHz.
            nc.gpsimd.memset(dummy[:], 0.0)
            warm_ps = ps_out.tile([L2, NI], F32, tag="out_ps")
            for _ in range(NWARM):
                nc.tensor.matmul(warm_ps[:], dummy[:, 0:L2], dummy[:])

            rhsp_round(0)
            load_asp(1)
            for p in range(1, PFP):
                load_pair(p)
            load_asp(2)
            load_asp(3)

            out_sb = None
            for b in range(BPC):
                if b % 2 == 0 and (b + 2 * PFP) // 2 < BPC // 2:
                    load_pair((b + 2 * PFP) // 2)
                t16, t8 = ctx_tiles[b // 2]

                g = next(
                    (i for i, (lo, _) in enumerate(ASP_PARTS) if lo == b), None
                )
                if g is not None:
                    sasp_burst(g)
                    if g + 1 < len(ASP_PARTS):
                        rhsp_round(g + 1)

                if b % OPACK == 0:
                    out_sb = out_pool.tile([L2, OPACK, L1], F16, tag="out")
                for c in range(NIC):
                    out_ps = ps_out.tile([L2, NI], F32, tag="out_ps")
                    for k in range(KC):
                        rhs = (
                            t16[:, b % 2, k, c * NI : (c + 1) * NI]
                            if k < KC16
                            else t8[:, b % 2, k - KC16, c * NI : (c + 1) * NI]
                        )
                        nc.tensor.matmul(
                            out_ps[:],
                            rhsp_t[:, b, k, :],
                            rhs,
                            start=(k == 0),
                            stop=(k == KC - 1),
                        )
                    # fp16(out_ps + s_asp[j]) -> SBUF; the rank-1 term rides
                    # the PSUM-drain copy as a per-partition scalar add.
                    nc.vector.tensor_scalar(
                        out_sb[:, b % OPACK, c * NI : (c + 1) * NI],
                        out_ps[:],
                        sasp_t[:, b : b + 1],
                        None,
                        ADD,
                    )

                if b % 2 == 1:
                    ctx_tiles.pop(b // 2)

                if b >= BPC - OPACK:
                    # tail: per-batch half-pair writes, parallel queues
                    q_issue(
                        outT[b // OPACK, :, b % OPACK : b % OPACK + 1, :],
                        out_sb[:, b % OPACK : b % OPACK + 1, :],
                        OUT_B // 2,
                    )
                elif b % OPACK == OPACK - 1:
                    q_issue(outT[b // OPACK], out_sb[:], OUT_B)

    nc.compile()
    return nc


_NC_CACHE = None


def _get_nc():
    global _NC_CACHE
    if _NC_CACHE is None:
        _NC_CACHE = build_kernel()
    return _NC_CACHE


def kernel(batch_size=None, ctx=None, asp=None, w_u=None, **run_kwargs):
    ctx = np.asarray(ctx, dtype=np.float32)
    asp = np.asarray(asp, dtype=np.float32)
    w_u = np.asarray(w_u, dtype=np.float32).reshape(-1)
    w1, w2, w3 = w_u[:H], w_u[H : 2 * H], w_u[2 * H :]

    # Sort contraction dims so the largest-|rhs'| dims stream as fp16 and
    # the rest as fp8 (permutation-invariant contraction).
    order = np.argsort(-(w3**2 + w1**2), kind="stable")
    ctx = ctx[:, :, order]
    asp = asp[:, :, order]
    w1, w2, w3 = w1[order], w2[order], w3[order]

    # Host-side layout transforms + dtype cast (partition-major so every
    # DMA descriptor is a long contiguous run; ctx packed in batch pairs).
    # ctxT*[b//2, p, b%2, k, i] = ctx[b, i, k*KP+p]
    cT = ctx.reshape(B // 2, 2, L1, KC, KP).transpose(0, 4, 1, 3, 2)
    cT = np.ascontiguousarray(cT)  # [B//2, KP, 2, KC, L1]
    ctxT16 = cT[:, :, :, :KC16].astype(np.float16)
    ctxT8 = np.ascontiguousarray(cT[:, :, :, KC16:]).astype(NP_F8)
    # aspT[p, b, k, j] = asp[b, j, k*KP+p]  (b local per core at slice time)
    aT = asp.reshape(B, L2, KC, KP).transpose(3, 0, 2, 1)  # [KP, B, KC, L2]
    aspT = np.ascontiguousarray(aT).astype(np.float16)
    # wc[p, 2*KC]: w1 chunk-cols | w3 (fp32, DVE scale/bias); w2c separate.
    wc = np.ascontiguousarray(
        np.concatenate([w1.reshape(KC, KP).T, w3.reshape(KC, KP).T], axis=1)
    ).astype(np.float32)
    w2c = np.ascontiguousarray(w2.reshape(KC, KP).T).astype(np.float16)

    nc = _get_nc()
    PPC = BPC // 2  # pairs per core
    in_maps = [
        {
            "ctxT16": np.ascontiguousarray(ctxT16[c * PPC : (c + 1) * PPC]),
            "ctxT8": np.ascontiguousarray(ctxT8[c * PPC : (c + 1) * PPC]),
            "aspT": aspT[:, c * BPC : (c + 1) * BPC],
            "wc": wc,
            "w2c": w2c,
        }
        for c in range(N_CORES)
    ]
    res = run_bass_kernel_spmd(
        nc, in_maps, core_ids=list(range(N_CORES)), **run_kwargs
    )
    outT = np.concatenate(
        [res.results[c]["outT"] for c in range(N_CORES)], axis=0
    ).astype(np.float32)  # [B//OPACK, L2, OPACK, L1]
    out = np.ascontiguousarray(
        outT.transpose(0, 2, 3, 1).reshape(B, L1, L2)
    )  # [B, L1, L2]
    if run_kwargs:
        return out, res
    return out
